# revision 6
# baseline (speedup 1.0000x reference)
"""Trainium2 Bass kernel: 2-layer dense transformer (B=4,S=1024,D=1024,H=16,FF=4096).

Sharding: 8 cores = 4 pairs (one batch element each) x 2-way sequence split
(512 queries per core). K/V are computed per-core over the full sequence from
a per-pair AllGather of x at the layer boundary (layer 0 gathers embeddings
for the full sequence directly, so only one collective total).

Self-contained: builds + compiles the Bass program on first call, caches the
jitted PJRT callable for repeat calls.
"""

import numpy as np

B, S, D, H, DH, FF, L, V = 4, 1024, 1024, 16, 64, 4096, 2, 32000
P = 128
TLOC = S // 2          # tokens (queries) per core
NC = 8
DC = D // P            # 8 d-chunks
MT = D // P            # 8 head-dim m-tiles (2 heads each)
FT = FF // P           # 32 ffn tiles
QB = TLOC // P         # 4 query blocks per core
KTT = S // P           # 8 key tiles
LN_EPS = 1e-5
NEG = -1.0e5           # mask bias (exp underflows to exactly 0)

# bias_sb column map ([P, 72] per layer)
BQ0, BK0, BO0, B20, BV0, B10 = 0, 8, 16, 24, 32, 40

_STATE = {}


def _build_program():
    import concourse.bass as bass
    import concourse.tile as tile
    from concourse import mybir, bacc

    f32 = mybir.dt.float32
    i32 = mybir.dt.int32
    AF = mybir.ActivationFunctionType
    OP = mybir.AluOpType

    nc = bacc.Bacc("TRN2", target_bir_lowering=False, debug=False, num_devices=NC)

    emb = nc.dram_tensor("emb", [V, D], f32, kind="ExternalInput")
    tokf = nc.dram_tensor("tokf", [S], i32, kind="ExternalInput")
    toko = nc.dram_tensor("toko", [TLOC], i32, kind="ExternalInput")
    maskb = nc.dram_tensor("maskb", [S], f32, kind="ExternalInput")
    possin = nc.dram_tensor("possin", [P, D], f32, kind="ExternalInput")
    ident_in = nc.dram_tensor("ident", [P, P], f32, kind="ExternalInput")
    ones_in = nc.dram_tensor("onespp", [P, P], f32, kind="ExternalInput")
    wq = nc.dram_tensor("wq", [L, DC, MT, P, P], f32, kind="ExternalInput")
    wk = nc.dram_tensor("wk", [L, DC, MT, P, P], f32, kind="ExternalInput")
    wv = nc.dram_tensor("wv", [L, DC, 2, P, 512], f32, kind="ExternalInput")
    wo = nc.dram_tensor("wo", [L, DC, MT, P, P], f32, kind="ExternalInput")
    w1 = nc.dram_tensor("w1", [L, DC, FT, P, P], f32, kind="ExternalInput")
    w2 = nc.dram_tensor("w2", [L, FT, DC, P, P], f32, kind="ExternalInput")
    biases = nc.dram_tensor("biases", [L, 72, P], f32, kind="ExternalInput")
    lnrep = nc.dram_tensor("lnrep", [L, 4, P, D], f32, kind="ExternalInput")
    out = nc.dram_tensor("out", [TLOC, D], f32, kind="ExternalOutput")

    ag_in = nc.dram_tensor("ag_in", [TLOC, D], f32)
    ag_out = nc.dram_tensor("ag_out", [S, D], f32)
    v_dram = nc.dram_tensor("v_dram", [KTT, P, D], f32)  # [ktile, tok, head*dh]
    groups = [[0, 1], [2, 3], [4, 5], [6, 7]]

    from contextlib import ExitStack

    with tile.TileContext(nc) as tc, ExitStack() as st:
        pp = st.enter_context(tc.tile_pool(name="persist", bufs=1))

        ident = pp.tile([P, P], f32, name="ident_sb")
        nc.sync.dma_start(out=ident[:], in_=ident_in[:])
        ones_t = pp.tile([P, P], f32, name="ones_sb")
        nc.sync.dma_start(out=ones_t[:], in_=ones_in[:])
        maskb_sb = pp.tile([P, KTT], f32, name="maskb_sb")
        nc.sync.dma_start(out=maskb_sb[:], in_=maskb[:].rearrange("(c p) -> p c", p=P))
        tokf_sb = pp.tile([P, KTT], i32, name="tokf_sb")
        nc.sync.dma_start(out=tokf_sb[:], in_=tokf[:].rearrange("(c p) -> p c", p=P))
        toko_sb = pp.tile([P, QB], i32, name="toko_sb")
        nc.sync.dma_start(out=toko_sb[:], in_=toko[:].rearrange("(c p) -> p c", p=P))
        bias_sb = []
        for l in range(L):
            bt = pp.tile([P, 72], f32, name=f"bias_sb{l}")
            nc.sync.dma_start(out=bt[:], in_=biases[l].rearrange("c p -> p c"))
            bias_sb.append(bt)

        # persistent activations
        xres = [pp.tile([P, D], f32, name=f"xres{t}") for t in range(QB)]
        xt_own = [pp.tile([P, TLOC], f32, name=f"xtown{j}") for j in range(DC)]
        xt_glob = [pp.tile([P, S], f32, name=f"xtglob{j}") for j in range(DC)]

        def tr_block(ps_pool, src_ap, dst_ap, use_dve=False):
            tp = ps_pool.tile([P, P], f32, tag="trp", bufs=2)
            nc.tensor.transpose(out=tp[:], in_=src_ap, identity=ident[:])
            if use_dve:
                nc.vector.tensor_copy(out=dst_ap, in_=tp[:])
            else:
                nc.scalar.copy(out=dst_ap, in_=tp[:])

        eps_sb = pp.tile([P, 1], f32, name="eps_sb")
        nc.vector.memset(eps_sb[:], LN_EPS)

        def layernorm(x_tiles, g_tile, b_tile, pool, out_tiles):
            for t in range(QB):
                stt = pool.tile([P, 2, 6], f32, tag="lnst", bufs=2)
                for sg in range(2):
                    nc.vector.bn_stats(out=stt[:, sg, :], in_=x_tiles[t][:, sg * 512:(sg + 1) * 512])
                mv = pool.tile([P, 2], f32, tag="lnmv", bufs=2)
                nc.vector.bn_aggr(out=mv[:], in_=stt[:])
                sd = pool.tile([P, 1], f32, tag="lnsd", bufs=2)
                nc.scalar.activation(out=sd[:], in_=mv[:, 1:2], func=AF.Sqrt, bias=eps_sb[:], scale=1.0)
                rstd = pool.tile([P, 1], f32, tag="lnr", bufs=2)
                nc.vector.reciprocal(out=rstd[:], in_=sd[:])
                nmr = pool.tile([P, 1], f32, tag="lnn", bufs=2)
                nc.vector.tensor_tensor(out=nmr[:], in0=mv[:, 0:1], in1=rstd[:], op=OP.mult)
                nc.vector.tensor_scalar_mul(nmr[:], nmr[:], -1.0)
                nc.scalar.activation(out=out_tiles[t][:], in_=x_tiles[t][:], func=AF.Identity,
                                     bias=nmr[:], scale=rstd[:])
                nc.vector.tensor_tensor(out=out_tiles[t][:], in0=out_tiles[t][:], in1=g_tile[:], op=OP.mult)
                nc.vector.tensor_tensor(out=out_tiles[t][:], in0=out_tiles[t][:], in1=b_tile[:], op=OP.add)

        # ---- init: embedding gather + positional + transposes ----
        with tc.tile_pool(name="init", bufs=1) as pi, \
             tc.tile_pool(name="init_ps", bufs=1, space="PSUM") as pips:
            possin_sb = pi.tile([P, D], f32, name="possin_sb")
            nc.sync.dma_start(out=possin_sb[:], in_=possin[:])
            for t in range(QB):
                nc.gpsimd.indirect_dma_start(
                    out=xres[t][:], out_offset=None, in_=emb[:],
                    in_offset=bass.IndirectOffsetOnAxis(ap=toko_sb[:, t:t + 1], axis=0))
                nc.vector.tensor_tensor(out=xres[t][:], in0=xres[t][:], in1=possin_sb[:], op=OP.add)
            for j in range(DC):
                for t in range(QB):
                    tr_block(pips, xres[t][:, j * P:(j + 1) * P], xt_own[j][:, t * P:(t + 1) * P])
            for t in range(KTT):
                xg = pi.tile([P, D], f32, tag="xg", bufs=2)
                nc.gpsimd.indirect_dma_start(
                    out=xg[:], out_offset=None, in_=emb[:],
                    in_offset=bass.IndirectOffsetOnAxis(ap=tokf_sb[:, t:t + 1], axis=0))
                nc.vector.tensor_tensor(out=xg[:], in0=xg[:], in1=possin_sb[:], op=OP.add)
                for j in range(DC):
                    tr_block(pips, xg[:, j * P:(j + 1) * P], xt_glob[j][:, t * P:(t + 1) * P],
                             use_dve=(j % 2 == 0))

        # ---- layers ----
        for l in range(L):
            bias_l = bias_sb[l]
            with tc.tile_pool(name=f"mid{l}", bufs=1) as pm:
                x1 = [pm.tile([P, D], f32, tag=f"x1_{t}", name=f"x1_{l}_{t}") for t in range(QB)]
                x1t = [pm.tile([P, TLOC], f32, tag=f"x1t_{j}", name=f"x1t_{l}_{j}") for j in range(DC)]

                with tc.tile_pool(name=f"attn{l}", bufs=1) as pa:
                    ln1g = pa.tile([P, D], f32, tag="ln1g", name=f"ln1g_{l}")
                    nc.sync.dma_start(out=ln1g[:], in_=lnrep[l, 0])
                    ln1b = pa.tile([P, D], f32, tag="ln1b", name=f"ln1b_{l}")
                    nc.sync.dma_start(out=ln1b[:], in_=lnrep[l, 1])

                    # P1: Q^T (own queries), all heads
                    qt = [pa.tile([P, TLOC], f32, tag=f"qt{g}", name=f"qt_{l}_{g}") for g in range(MT)]
                    with tc.tile_pool(name=f"ps_q{l}", bufs=1, space="PSUM") as psq:
                        for g in range(MT):
                            ps = psq.tile([P, TLOC], f32, tag="proj", bufs=2)
                            for kc in range(DC):
                                wqt = pa.tile([P, P], f32, tag="wqt", bufs=4)
                                nc.sync.dma_start(out=wqt[:], in_=wq[l, kc, g])
                                nc.tensor.matmul(ps[:], lhsT=wqt[:], rhs=xt_own[kc][:],
                                                 start=(kc == 0), stop=(kc == DC - 1))
                            nc.scalar.activation(out=qt[g][:], in_=ps[:], func=AF.Identity,
                                                 bias=bias_l[:, BQ0 + g:BQ0 + g + 1], scale=1.0)

                        # P3: V (token-major, all heads, full seq) -> v_dram
                        for ktg in range(2):
                            for half in range(2):
                                pss = [psq.tile([P, 512], f32, tag="vproj", bufs=4, name=f"vps_{l}_{ktg}_{half}_{i2}") for i2 in range(4)]
                                for kc in range(DC):
                                    wvt = pa.tile([P, 512], f32, tag="wvt", bufs=3)
                                    nc.sync.dma_start(out=wvt[:], in_=wv[l, kc, half])
                                    for i, kt in enumerate(range(ktg * 4, ktg * 4 + 4)):
                                        nc.tensor.matmul(pss[i][:],
                                                         lhsT=xt_glob[kc][:, kt * P:(kt + 1) * P],
                                                         rhs=wvt[:], start=(kc == 0), stop=(kc == DC - 1))
                                for i, kt in enumerate(range(ktg * 4, ktg * 4 + 4)):
                                    vtmp = pa.tile([P, 512], f32, tag="vtmp", bufs=3)
                                    nc.scalar.copy(out=vtmp[:], in_=pss[i][:])
                                    nc.sync.dma_start(out=v_dram[kt, :, half * 512:(half + 1) * 512],
                                                      in_=vtmp[:])

                    # P4: attention, 2 heads per group
                    ot = [pa.tile([P, TLOC], f32, tag=f"ot{g}", name=f"ot_{l}_{g}") for g in range(MT)]
                    with tc.tile_pool(name=f"ps_a{l}", bufs=1, space="PSUM") as psa:
                        for g in range(MT):
                            # K^T for heads 2g, 2g+1 over full seq
                            ktg_t = pa.tile([P, S], f32, tag="ktg", bufs=2)
                            for half in range(2):
                                ps = psa.tile([P, 512], f32, tag="kproj", bufs=2)
                                for kc in range(DC):
                                    wkt = pa.tile([P, P], f32, tag="wkt", bufs=4)
                                    nc.sync.dma_start(out=wkt[:], in_=wk[l, kc, g])
                                    nc.tensor.matmul(ps[:], lhsT=wkt[:],
                                                     rhs=xt_glob[kc][:, half * 512:(half + 1) * 512],
                                                     start=(kc == 0), stop=(kc == DC - 1))
                                nc.scalar.activation(out=ktg_t[:, half * 512:(half + 1) * 512], in_=ps[:],
                                                     func=AF.Identity, bias=bias_l[:, BK0 + g:BK0 + g + 1],
                                                     scale=1.0)
                            ops_ = psa.tile([P, TLOC], f32, tag="o", bufs=1)
                            for hh in range(2):
                                h = 2 * g + hh
                                pb = hh * 64
                                dps = psa.tile([1, TLOC], f32, tag="dn", bufs=2)
                                for kt in range(KTT):
                                    sps = psa.tile([P, TLOC], f32, tag="sc", bufs=2)
                                    nc.tensor.matmul(sps[:],
                                                     lhsT=ktg_t[pb:pb + 64, kt * P:(kt + 1) * P],
                                                     rhs=qt[g][pb:pb + 64, :], start=True, stop=True)
                                    at = pa.tile([P, TLOC], f32, tag="attnT", bufs=4)
                                    nc.scalar.activation(out=at[:], in_=sps[:], func=AF.Exp,
                                                         bias=maskb_sb[:, kt:kt + 1], scale=0.125)
                                    vh = pa.tile([P, DH], f32, tag="vh", bufs=8)
                                    nc.sync.dma_start(out=vh[:], in_=v_dram[kt, :, h * DH:(h + 1) * DH])
                                    nc.tensor.matmul(ops_[pb:pb + 64, :], lhsT=vh[:], rhs=at[:],
                                                     start=(kt == 0), stop=(kt == KTT - 1))
                                    nc.tensor.matmul(dps[:], lhsT=ones_t[:, 0:1], rhs=at[:],
                                                     start=(kt == 0), stop=(kt == KTT - 1))
                                recip = pa.tile([1, TLOC], f32, tag="recip", bufs=2)
                                nc.vector.reciprocal(out=recip[:], in_=dps[:])
                                bps = psa.tile([P, TLOC], f32, tag="bc", bufs=1)
                                nc.tensor.matmul(bps[pb:pb + 64, :], lhsT=ones_t[0:1, pb:pb + 64],
                                                 rhs=recip[:], start=True, stop=True)
                                rb_sb = pa.tile([P, TLOC], f32, tag="rbsb", bufs=2)
                                nc.scalar.copy(out=rb_sb[pb:pb + 64, :], in_=bps[pb:pb + 64, :])
                                nc.vector.tensor_tensor(out=ot[g][pb:pb + 64, :], in0=ops_[pb:pb + 64, :],
                                                        in1=rb_sb[pb:pb + 64, :], op=OP.mult)
                                nc.scalar.activation(out=ot[g][pb:pb + 64, :], in_=ot[g][pb:pb + 64, :],
                                                     func=AF.Identity,
                                                     bias=bias_l[pb:pb + 64, BV0 + g:BV0 + g + 1], scale=1.0)

                    # P5: Wo + residual -> x1 ; LN1 in place
                    with tc.tile_pool(name=f"ps_m{l}", bufs=1, space="PSUM") as psm:
                        for dt in range(DC):
                            ps = psm.tile([P, TLOC], f32, tag="proj", bufs=2)
                            for hc in range(DC):
                                wot = pa.tile([P, P], f32, tag="wot", bufs=4)
                                nc.sync.dma_start(out=wot[:], in_=wo[l, hc, dt])
                                nc.tensor.matmul(ps[:], lhsT=wot[:], rhs=ot[hc][:],
                                                 start=(hc == 0), stop=(hc == DC - 1))
                            ao = pa.tile([P, TLOC], f32, tag="ao", bufs=3)
                            nc.scalar.activation(out=ao[:], in_=ps[:], func=AF.Identity,
                                                 bias=bias_l[:, BO0 + dt:BO0 + dt + 1], scale=1.0)
                            for qb in range(QB):
                                tp = psm.tile([P, P], f32, tag="trp", bufs=2)
                                nc.tensor.transpose(out=tp[:], in_=ao[:, qb * P:(qb + 1) * P],
                                                    identity=ident[:])
                                nc.vector.tensor_tensor(out=x1[qb][:, dt * P:(dt + 1) * P], in0=tp[:],
                                                        in1=xres[qb][:, dt * P:(dt + 1) * P], op=OP.add)
                        layernorm(x1, ln1g, ln1b, pa, x1)
                        # P6: x1 -> x1t (feature-major)
                        for j in range(DC):
                            for t in range(QB):
                                tr_block(psm, x1[t][:, j * P:(j + 1) * P],
                                         x1t[j][:, t * P:(t + 1) * P], use_dve=(t % 2 == 0))

                # P7: FFN
                with tc.tile_pool(name=f"ffn{l}", bufs=1) as pf, \
                     tc.tile_pool(name=f"ps_f{l}", bufs=1, space="PSUM") as psf:
                    ln2g = pf.tile([P, D], f32, tag="ln2g", name=f"ln2g_{l}")
                    nc.sync.dma_start(out=ln2g[:], in_=lnrep[l, 2])
                    ln2b = pf.tile([P, D], f32, tag="ln2b", name=f"ln2b_{l}")
                    nc.sync.dma_start(out=ln2b[:], in_=lnrep[l, 3])
                    h1 = []
                    for ft in range(FT):
                        ps = psf.tile([P, TLOC], f32, tag="proj", bufs=2)
                        for kc in range(DC):
                            w1t = pf.tile([P, P], f32, tag="w1t", bufs=4)
                            nc.sync.dma_start(out=w1t[:], in_=w1[l, kc, ft])
                            nc.tensor.matmul(ps[:], lhsT=w1t[:], rhs=x1t[kc][:],
                                             start=(kc == 0), stop=(kc == DC - 1))
                        ht = pf.tile([P, TLOC], f32, tag="h1", bufs=FT)
                        nc.scalar.activation(out=ht[:], in_=ps[:], func=AF.Relu,
                                             bias=bias_l[:, B10 + ft:B10 + ft + 1], scale=1.0)
                        h1.append(ht)
                    x2 = [pf.tile([P, D], f32, tag=f"x2_{t}", name=f"x2_{l}_{t}") for t in range(QB)]
                    for dt in range(DC):
                        ps = psf.tile([P, TLOC], f32, tag="acc", bufs=2)
                        for fc in range(FT):
                            w2t = pf.tile([P, P], f32, tag="w2t", bufs=4)
                            nc.sync.dma_start(out=w2t[:], in_=w2[l, fc, dt])
                            nc.tensor.matmul(ps[:], lhsT=w2t[:], rhs=h1[fc][:],
                                             start=(fc == 0), stop=(fc == FT - 1))
                        h2 = pf.tile([P, TLOC], f32, tag="h2", bufs=3)
                        nc.scalar.activation(out=h2[:], in_=ps[:], func=AF.Identity,
                                             bias=bias_l[:, B20 + dt:B20 + dt + 1], scale=1.0)
                        for qb in range(QB):
                            tp = psf.tile([P, P], f32, tag="trp", bufs=2)
                            nc.tensor.transpose(out=tp[:], in_=h2[:, qb * P:(qb + 1) * P],
                                                identity=ident[:])
                            nc.vector.tensor_tensor(out=x2[qb][:, dt * P:(dt + 1) * P], in0=tp[:],
                                                    in1=x1[qb][:, dt * P:(dt + 1) * P], op=OP.add)
                    layernorm(x2, ln2g, ln2b, pf, xres)

            # P8: exchange x between pair halves (except after last layer)
            if l < L - 1:
                with tc.tile_pool(name=f"exch{l}", bufs=1) as px, \
                     tc.tile_pool(name=f"ps_x{l}", bufs=1, space="PSUM") as psx:
                    for t in range(QB):
                        nc.sync.dma_start(out=ag_in[t * P:(t + 1) * P, :], in_=xres[t][:])
                    nc.gpsimd.collective_compute(
                        "AllGather", mybir.AluOpType.bypass, replica_groups=groups,
                        ins=[ag_in[:]], outs=[ag_out[:]])
                    for j in range(DC):
                        for t in range(QB):
                            tr_block(psx, xres[t][:, j * P:(j + 1) * P],
                                     xt_own[j][:, t * P:(t + 1) * P])
                    for t in range(KTT):
                        xg2 = px.tile([P, D], f32, tag="xg2", bufs=2)
                        nc.sync.dma_start(out=xg2[:], in_=ag_out[t * P:(t + 1) * P, :])
                        for j in range(DC):
                            tr_block(psx, xg2[:, j * P:(j + 1) * P],
                                     xt_glob[j][:, t * P:(t + 1) * P], use_dve=(j % 2 == 0))
        # final output
        for t in range(QB):
            nc.sync.dma_start(out=out[t * P:(t + 1) * P, :], in_=xres[t][:])

    nc.compile()
    return nc


def _make_runner():
    """Build program, compile via PJRT once, return callable(in_maps)->results list."""
    import jax
    import jax.numpy as jnp
    import numpy as _np
    from jax.sharding import Mesh, PartitionSpec, NamedSharding
    from jax.experimental.shard_map import shard_map
    from concourse import bass2jax, mybir

    nc = _build_program()
    bass2jax.install_neuronx_cc_hook()

    partition_name = nc.partition_id_tensor.name if nc.partition_id_tensor else None
    in_names, out_names, out_avals, zero_shapes = [], [], [], []
    for alloc in nc.m.functions[0].allocations:
        if not isinstance(alloc, mybir.MemoryLocationSet):
            continue
        name = alloc.memorylocations[0].name
        if alloc.kind == "ExternalInput":
            if name != partition_name:
                in_names.append(name)
        elif alloc.kind == "ExternalOutput":
            shape = tuple(alloc.tensor_shape)
            dtype = mybir.dt.np(alloc.dtype)
            out_names.append(name)
            out_avals.append(jax.core.ShapedArray(shape, dtype))
            zero_shapes.append((shape, dtype))
    n_params = len(in_names)
    n_outs = len(out_avals)
    all_in_names = list(in_names) + list(out_names)
    if partition_name is not None:
        all_in_names.append(partition_name)
    donate = tuple(range(n_params, n_params + n_outs))

    def _body(*args):
        operands = list(args)
        if partition_name is not None:
            operands.append(bass2jax.partition_id_tensor())
        outs = bass2jax._bass_exec_p.bind(
            *operands,
            out_avals=tuple(out_avals),
            in_names=tuple(all_in_names),
            out_names=tuple(out_names),
            lowering_input_output_aliases=(),
            sim_require_finite=True,
            sim_require_nnan=True,
            nc=nc,
        )
        return tuple(outs)

    devices = jax.devices()[:NC]
    mesh = Mesh(_np.asarray(devices), ("core",))
    in_specs = (PartitionSpec("core"),) * (n_params + n_outs)
    out_specs = (PartitionSpec("core"),) * n_outs
    sharded = jax.jit(
        shard_map(_body, mesh=mesh, in_specs=in_specs, out_specs=out_specs,
                  check_rep=False),
        donate_argnums=donate, keep_unused=True)

    zsharding = [NamedSharding(mesh, PartitionSpec("core")) for _ in range(n_outs)]

    _zeros_jit = jax.jit(
        lambda: tuple(jnp.zeros((NC * sh[0], *sh[1:]), dt) for sh, dt in zero_shapes),
        out_shardings=tuple(zsharding))

    def mk_zeros():
        return list(_zeros_jit())

    def run(in_maps, device_inputs=None):
        if device_inputs is None:
            device_inputs = put_inputs(in_maps)
        zeros = mk_zeros()
        out_arrs = sharded(*device_inputs, *zeros)
        res = []
        for c in range(NC):
            res.append({
                name: _np.asarray(out_arrs[i]).reshape(NC, *out_avals[i].shape)[c]
                for i, name in enumerate(out_names)
            })
        return res

    def put_inputs(in_maps):
        ins = []
        for i, name in enumerate(in_names):
            concat = _np.concatenate([_np.asarray(m[name]) for m in in_maps], axis=0)
            ins.append(jax.device_put(concat, NamedSharding(mesh, PartitionSpec("core"))))
        return ins

    run.put_inputs = put_inputs
    run.mk_zeros = mk_zeros
    run.sharded = sharded
    run.in_names = in_names
    run.out_names = out_names
    run.nc = nc
    return run


def _get_runner():
    if "run" not in _STATE:
        _STATE["run"] = _make_runner()
    return _STATE["run"]


def _tile2(w, pdim, mdim):
    """[A, B] -> [A//pdim, B//mdim, pdim, mdim] contiguous tiles."""
    a, b = w.shape
    return np.ascontiguousarray(
        w.reshape(a // pdim, pdim, b // mdim, mdim).transpose(0, 2, 1, 3))


def prep_inputs(tokens, mask, emb, Wq, bq, Wk, bk, Wv, bv, Wo, bo,
                ln1_g, ln1_b, W1, b1, W2, b2, ln2_g, ln2_b):
    f32 = np.float32
    tokens = np.asarray(tokens).astype(np.int32)
    mask = np.asarray(mask)
    maskb = np.where(mask == 0, f32(NEG), f32(0.0)).astype(f32)

    emb = np.ascontiguousarray(np.asarray(emb, dtype=f32))
    possin = np.tile(np.sin(np.arange(D, dtype=f32))[None, :], (P, 1))
    ident = np.eye(P, dtype=f32)
    onespp = np.ones((P, P), dtype=f32)

    def cat_heads(w):  # [L, H, D, DH] -> [L, D, H*DH]
        return np.asarray(w, f32).transpose(0, 2, 1, 3).reshape(L, D, H * DH)

    wq_t = np.stack([_tile2(w, P, P) for w in cat_heads(Wq)])
    wk_t = np.stack([_tile2(w, P, P) for w in cat_heads(Wk)])
    wv_t = np.stack([_tile2(w, P, 512) for w in cat_heads(Wv)])
    wo_t = np.stack([_tile2(np.asarray(w, f32), P, P) for w in np.asarray(Wo, f32)])
    w1_t = np.stack([_tile2(np.asarray(w, f32), P, P) for w in np.asarray(W1, f32)])
    w2_t = np.stack([_tile2(np.asarray(w, f32), P, P) for w in np.asarray(W2, f32)])

    biases = np.zeros((L, 72, P), f32)
    for l in range(L):
        biases[l, BQ0:BQ0 + 8] = np.asarray(bq, f32)[l].reshape(H * DH).reshape(8, P)
        biases[l, BK0:BK0 + 8] = np.asarray(bk, f32)[l].reshape(H * DH).reshape(8, P)
        biases[l, BO0:BO0 + 8] = np.asarray(bo, f32)[l].reshape(8, P)
        biases[l, B20:B20 + 8] = np.asarray(b2, f32)[l].reshape(8, P)
        biases[l, BV0:BV0 + 8] = np.asarray(bv, f32)[l].reshape(H * DH).reshape(8, P)
        biases[l, B10:B10 + 32] = np.asarray(b1, f32)[l].reshape(32, P)

    lnrep = np.zeros((L, 4, P, D), f32)
    for l in range(L):
        lnrep[l, 0] = np.tile(np.asarray(ln1_g, f32)[l][None, :], (P, 1))
        lnrep[l, 1] = np.tile(np.asarray(ln1_b, f32)[l][None, :], (P, 1))
        lnrep[l, 2] = np.tile(np.asarray(ln2_g, f32)[l][None, :], (P, 1))
        lnrep[l, 3] = np.tile(np.asarray(ln2_b, f32)[l][None, :], (P, 1))

    shared = dict(emb=emb, possin=possin, ident=ident, onespp=onespp,
                  wq=wq_t, wk=wk_t, wv=wv_t, wo=wo_t, w1=w1_t, w2=w2_t,
                  biases=biases, lnrep=lnrep)
    in_maps = []
    for c in range(NC):
        b, h = c // 2, c % 2
        m = dict(shared)
        m["tokf"] = tokens[b]
        m["toko"] = tokens[b, h * TLOC:(h + 1) * TLOC]
        m["maskb"] = maskb[b]
        in_maps.append(m)
    return in_maps


def kernel(**inputs):
    run = _get_runner()
    in_maps = prep_inputs(**inputs)
    results = run(in_maps)
    out = np.empty((B, S, D), np.float32)
    for c in range(NC):
        out[c // 2, (c % 2) * TLOC:((c % 2) + 1) * TLOC, :] = results[c]["out"]
    return out
